# revision 6
# baseline (speedup 1.0000x reference)
"""Multi-head attention (unfused) for one TRN2 chip (8 NeuronCores).

Sharding: 2 batches x 4 head-groups (4 heads each) = 8 cores.
Core c handles batch b = c // 4, head-group g = c % 4 (heads 4g..4g+3,
i.e. rows 256g..256g+255 of the QKV projections).

Host side pre-transposes activations to [E, S] ("xT") and weights so the
device kernel never transposes anything:
  qT = WqT.T @ xqT + bq          [256, S]   (lhsT=WqT tile, rhs=xqT tile)
  kT = WkT.T @ xkT + bk          [256, S]
  v  = xvT.T @ WvT + bv          [S, 256]   (lhsT=xvT tile, rhs=WvT tile)
  per head h:
    scoresT = kT_h.T @ qT_h      [S_k, S_q] (layout: keys on partitions)
    expT    = exp(scoresT/8)     (ScalarE, scale fused)
    pv      = [v_h | 1].T @ expT [65, S_q]  (row 64 = softmax denominator)
    attnT_h = pv[0:64] * (1/pv[64]) (broadcast via stride-0 DMA)
  outT_partial = WoT.T @ attnT   [E, S]
Host sums the 4 partials per batch, adds bo, transposes back.

All matmuls run with float32r operands (full-rate PE) accumulating fp32.
"""

import os
import sys

sys.path.insert(0, "/opt/trn_rl_repo")

import numpy as np

import concourse.bacc as bacc
import concourse.bass as bass
import concourse.mybir as mybir
import concourse.tile as tile
from concourse import library_config

F32 = mybir.dt.float32
F32R = mybir.dt.float32r

S = 2048          # sequence length (keys and queries)
E = 1024          # embed dim
P = 256           # projection rows per core (4 heads x 64)
D = 64            # head dim
HL = 4            # heads per core
NCORES = 8

EKT = E // 128    # 8 contraction k-tiles for projections
MT = P // 128     # 2 m-tiles for kT/qT
NSC = S // 512    # 4 s-chunks / q-chunks
NKT = S // 128    # 16 key tiles

ROW_PACK = os.environ.get("KB_ROW_PACK", "1") == "1"


def _r(ap):
    return ap.bitcast(F32R)


def build_nc():
    nc = bacc.Bacc(trn_type="TRN2", debug=False, num_devices=NCORES,
                   enable_asserts=False)

    xq = nc.dram_tensor("xq", [E, S], F32R, kind="ExternalInput")
    xk = nc.dram_tensor("xk", [E, S], F32R, kind="ExternalInput")
    xv = nc.dram_tensor("xv", [E, S], F32R, kind="ExternalInput")
    wq = nc.dram_tensor("wq", [E, P], F32R, kind="ExternalInput")
    wk = nc.dram_tensor("wk", [E, P], F32R, kind="ExternalInput")
    wv = nc.dram_tensor("wv", [E, HL * (D + 1)], F32R, kind="ExternalInput")
    wo = nc.dram_tensor("wo", [P, E], F32R, kind="ExternalInput")
    bq = nc.dram_tensor("bq", [128, MT], F32, kind="ExternalInput")
    bk = nc.dram_tensor("bk", [128, MT], F32, kind="ExternalInput")
    bv = nc.dram_tensor("bv", [1, HL * (D + 1)], F32R, kind="ExternalInput")
    ones = nc.dram_tensor("ones", [1, 128], F32R, kind="ExternalInput")
    out = nc.dram_tensor("out", [E, S], F32, kind="ExternalOutput")

    with tile.TileContext(nc) as tc:
        with (
            tc.tile_pool(name="consts", bufs=1) as cpool,
            tc.tile_pool(name="xstage", bufs=3) as xpool,
            tc.tile_pool(name="kqv", bufs=1) as kqv_pool,
            tc.tile_pool(name="exp", bufs=6) as exp_pool,
            tc.tile_pool(name="attnsb", bufs=2) as attnsb_pool,
            tc.tile_pool(name="small", bufs=4) as small_pool,
            tc.tile_pool(name="outstage", bufs=4) as out_pool,
            tc.tile_pool(name="ps", bufs=4, space=bass.MemorySpace.PSUM) as ps_pool,
            tc.tile_pool(name="psattn", bufs=4, space=bass.MemorySpace.PSUM) as psa_pool,
        ):
            # ---- constants ----
            wq_sb = cpool.tile([128, EKT, P], F32R, tag="wq")
            wk_sb = cpool.tile([128, EKT, P], F32R, tag="wk")
            wv_sb = cpool.tile([128, EKT, HL * (D + 1)], F32R, tag="wv")
            wo_sb = cpool.tile([128, MT, E], F32R, tag="wo")
            bq_sb = cpool.tile([128, MT], F32, tag="bq")
            bk_sb = cpool.tile([128, MT], F32, tag="bk")
            bv_sb = cpool.tile([1, HL * (D + 1)], F32R, tag="bv")
            ones_row = cpool.tile([1, 128], F32R, tag="ones")

            nc.sync.dma_start(wq_sb[:], wq.ap().rearrange("(a p) m -> p a m", p=128))
            nc.sync.dma_start(wk_sb[:], wk.ap().rearrange("(a p) m -> p a m", p=128))
            nc.sync.dma_start(wv_sb[:], wv.ap().rearrange("(a p) m -> p a m", p=128))
            nc.sync.dma_start(wo_sb[:], wo.ap().rearrange("(a p) m -> p a m", p=128))
            nc.sync.dma_start(bq_sb[:], bq.ap())
            nc.sync.dma_start(bk_sb[:], bk.ap())
            nc.sync.dma_start(bv_sb[:], bv.ap())
            nc.sync.dma_start(ones_row[:], ones.ap())
            nc.gpsimd.load_library(library_config.attn)

            kT_sb = kqv_pool.tile([128, MT, S], F32R, tag="kT")
            qT_sb = kqv_pool.tile([128, MT, S], F32R, tag="qT")
            v_sb = kqv_pool.tile([128, NKT, HL, D + 1], F32R, tag="v")

            def load_chunk(x, sc2, tag):
                t = xpool.tile([128, EKT, 512], F32R, tag="x", name="x_" + tag)
                nc.sync.dma_start(
                    t[:], x.ap()[:, sc2 * 512:(sc2 + 1) * 512]
                    .rearrange("(a p) s -> p a s", p=128))
                return t

            def proj_kq(x_t, w_sb, b_sb, dst_sb, sc2):
                # dst[:, mt, sc2*512:...] = w.T @ x + b
                for mt in range(MT):
                    ps = ps_pool.tile([128, 512], F32, tag="mm")
                    for ekt in range(EKT):
                        nc.tensor.matmul(
                            ps[:],
                            _r(w_sb[:, ekt, mt * 128:(mt + 1) * 128]),
                            _r(x_t[:, ekt, :]),
                            start=(ekt == 0), stop=(ekt == EKT - 1))
                    nc.vector.tensor_scalar_add(
                        dst_sb[:, mt, sc2 * 512:(sc2 + 1) * 512],
                        ps[:], b_sb[:, mt:mt + 1])

            def proj_v(xv_t, sc2):
                # v[st, :] = xv.T @ wv + bv, st-tiles of 128 rows
                for sti in range(4):
                    st = sc2 * 4 + sti
                    PV = HL * (D + 1)
                    ps = ps_pool.tile([128, 512], F32, tag="mm")
                    for ekt in range(EKT):
                        nc.tensor.matmul(
                            ps[:, 0:PV],
                            _r(xv_t[:, ekt, sti * 128:(sti + 1) * 128]),
                            _r(wv_sb[:, ekt, :]),
                            start=(ekt == 0), stop=False)
                    nc.tensor.matmul(
                        ps[:, 0:PV], _r(ones_row[:]), _r(bv_sb[:]),
                        start=False, stop=True)
                    nc.vector.tensor_copy(
                        v_sb[:, st, :, :],
                        ps[:, 0:PV].rearrange("p (h d) -> p h d", h=HL))

            # ---- phase B: k, v projections over all s-chunks ----
            for sc2 in range(NSC):
                xk_t = load_chunk(xk, sc2, "xk")
                xv_t = load_chunk(xv, sc2, "xv")
                proj_kq(xk_t, wk_sb, bk_sb, kT_sb, sc2)
                proj_v(xv_t, sc2)

            # ---- phase C: q projection + attention + out-proj, per q-chunk ----
            for sc in range(NSC):
                xq_t = load_chunk(xq, sc, "xq")
                proj_kq(xq_t, wq_sb, bq_sb, qT_sb, sc)

                attn_sb = attnsb_pool.tile([128, MT, 512], F32R, tag="attn_sb")
                for hp in range(2):
                    attn_ps = {}
                    for i in range(2):
                        h = 2 * hp + i
                        attn_ps[h] = psa_pool.tile([D + 1, 512], F32, tag="pv",
                                                   name=f"pv_{sc}_{h}")
                    for kt in range(NKT):
                        for i in range(2):
                            h = 2 * hp + i
                            lo, hi = i * 64, (i + 1) * 64
                            s_ps = ps_pool.tile([128, 512], F32, tag="mm")
                            nc.tensor.matmul(
                                s_ps[:],
                                _r(kT_sb[lo:hi, hp, kt * 128:(kt + 1) * 128]),
                                _r(qT_sb[lo:hi, hp, sc * 512:(sc + 1) * 512]),
                                start=True, stop=True,
                                tile_position=(lo, 0) if ROW_PACK else None)
                            exp_t = exp_pool.tile([128, 512], F32R, tag="exp")
                            nc.scalar.activation(
                                exp_t[:], s_ps[:],
                                mybir.ActivationFunctionType.Exp,
                                scale=0.125)
                            nc.tensor.matmul(
                                attn_ps[h][:],
                                _r(v_sb[:, kt, h, :]),
                                _r(exp_t[:]),
                                start=(kt == 0), stop=(kt == NKT - 1))
                    for i in range(2):
                        h = 2 * hp + i
                        rc = small_pool.tile([1, 512], F32, tag="recip")
                        nc.vector.reciprocal(rc[:], attn_ps[h][D:D + 1, :])
                        bc = small_pool.tile([D, 512], F32, tag="bc")
                        nc.gpsimd.partition_broadcast(bc[:], rc[:])
                        nc.vector.tensor_mul(
                            attn_sb[(h % 2) * 64:(h % 2 + 1) * 64, h // 2, :],
                            attn_ps[h][0:D, :], bc[:])

                for mt in range(E // 128):
                    ps_o = ps_pool.tile([128, 512], F32, tag="mm")
                    for kt2 in range(MT):
                        nc.tensor.matmul(
                            ps_o[:],
                            _r(wo_sb[:, kt2, mt * 128:(mt + 1) * 128]),
                            _r(attn_sb[:, kt2, :]),
                            start=(kt2 == 0), stop=(kt2 == MT - 1))
                    ot = out_pool.tile([128, 512], F32, tag="ot")
                    nc.vector.tensor_copy(ot[:], ps_o[:])
                    nc.sync.dma_start(
                        out.ap()[mt * 128:(mt + 1) * 128, sc * 512:(sc + 1) * 512],
                        ot[:])

    nc.compile()
    return nc


_NC_CACHE = None


def _get_nc():
    global _NC_CACHE
    if _NC_CACHE is None:
        _NC_CACHE = build_nc()
    return _NC_CACHE


def make_in_maps(key, query, value, Wk, bk, Wq, bq, Wv, bv, Wo, bo):
    key = np.asarray(key, np.float32)
    query = np.asarray(query, np.float32)
    value = np.asarray(value, np.float32)
    in_maps = []
    xqT = [np.ascontiguousarray(query[b].T) for b in range(2)]
    xkT = [np.ascontiguousarray(key[b].T) for b in range(2)]
    xvT = [np.ascontiguousarray(value[b].T) for b in range(2)]
    for c in range(NCORES):
        b, g = divmod(c, 4)
        rows = slice(g * P, (g + 1) * P)
        wv_slice = np.asarray(Wv, np.float32)[rows].T  # [E, 256]
        bv_slice = np.asarray(bv, np.float32)[rows]
        wv_ext = np.zeros((E, HL * (D + 1)), np.float32)
        bv_ext = np.zeros((1, HL * (D + 1)), np.float32)
        for h in range(HL):
            wv_ext[:, h * (D + 1):h * (D + 1) + D] = wv_slice[:, h * D:(h + 1) * D]
            bv_ext[0, h * (D + 1):h * (D + 1) + D] = bv_slice[h * D:(h + 1) * D]
            bv_ext[0, h * (D + 1) + D] = 1.0
        in_maps.append({
            "xq": xqT[b],
            "xk": xkT[b],
            "xv": xvT[b],
            "wq": np.ascontiguousarray(np.asarray(Wq, np.float32)[rows].T),
            "wk": np.ascontiguousarray(np.asarray(Wk, np.float32)[rows].T),
            "wv": wv_ext,
            "wo": np.ascontiguousarray(np.asarray(Wo, np.float32)[:, rows].T),
            "bq": np.ascontiguousarray(
                np.asarray(bq, np.float32)[rows].reshape(MT, 128).T),
            "bk": np.ascontiguousarray(
                np.asarray(bk, np.float32)[rows].reshape(MT, 128).T),
            "bv": bv_ext,
            "ones": np.ones((1, 128), np.float32),
        })
    return in_maps


def assemble(results, bo):
    bo = np.asarray(bo, np.float32)
    out = np.empty((2, S, E), np.float32)
    for b in range(2):
        acc = results[4 * b]["out"].astype(np.float32).copy()
        for g in range(1, 4):
            acc += results[4 * b + g]["out"]
        out[b] = acc.T + bo[None, :]
    return out


def kernel(key, query, value, Wk, bk, Wq, bq, Wv, bv, Wo, bo):
    from concourse.bass_utils import run_bass_kernel_spmd

    nc = _get_nc()
    in_maps = make_in_maps(key, query, value, Wk, bk, Wq, bq, Wv, bv, Wo, bo)
    trace = os.environ.get("KB_TRACE", "0") == "1"
    kwargs = {}
    if trace:
        kwargs["trace"] = True
        kwargs["trace_cores"] = list(range(NCORES))
    res = run_bass_kernel_spmd(nc, in_maps, core_ids=list(range(NCORES)), **kwargs)
    if trace:
        kernel.last_results = res
    return assemble(res.results, bo)


# revision 8
# speedup vs baseline: 1.1375x; 1.1375x over previous
"""Multi-head attention (unfused) for one TRN2 chip (8 NeuronCores).

Sharding: 2 batches x 4 head-groups (4 heads each) = 8 cores.
Core c handles batch b = c // 4, head-group g = c % 4 (heads 4g..4g+3,
i.e. rows 256g..256g+255 of the QKV projections).

Host side pre-transposes activations to [E, S] ("xT") and weights so the
device kernel never transposes anything:
  qT = WqT.T @ xqT + bq          [256, S]   (lhsT=WqT tile, rhs=xqT tile)
  kT = WkT.T @ xkT + bk          [256, S]
  v  = xvT.T @ WvT + bv          [S, 256]   (lhsT=xvT tile, rhs=WvT tile)
  per head h:
    scoresT = kT_h.T @ qT_h      [S_k, S_q] (layout: keys on partitions)
    expT    = exp(scoresT/8)     (ScalarE, scale fused)
    pv      = [v_h | 1].T @ expT [65, S_q]  (row 64 = softmax denominator)
    attnT_h = pv[0:64] * (1/pv[64]) (broadcast via stride-0 DMA)
  outT_partial = WoT.T @ attnT   [E, S]
Host sums the 4 partials per batch, adds bo, transposes back.

All matmuls run with float32r operands (full-rate PE) accumulating fp32.
"""

import os
import sys

sys.path.insert(0, "/opt/trn_rl_repo")

import numpy as np

import concourse.bacc as bacc
import concourse.bass as bass
import concourse.mybir as mybir
import concourse.tile as tile
from concourse import library_config

F32 = mybir.dt.float32
F32R = mybir.dt.float32r

S = 2048          # sequence length (keys and queries)
E = 1024          # embed dim
P = 256           # projection rows per core (4 heads x 64)
D = 64            # head dim
HL = 4            # heads per core
NCORES = 8

EKT = E // 128    # 8 contraction k-tiles for projections
MT = P // 128     # 2 m-tiles for kT/qT
NSC = S // 512    # 4 s-chunks / q-chunks
NKT = S // 128    # 16 key tiles

ROW_PACK = os.environ.get("KB_ROW_PACK", "1") == "1"


def _r(ap):
    return ap.bitcast(F32R)


def build_nc():
    nc = bacc.Bacc(trn_type="TRN2", debug=False, num_devices=NCORES,
                   enable_asserts=False)

    xq = nc.dram_tensor("xq", [E, S], F32R, kind="ExternalInput")
    xk = nc.dram_tensor("xk", [E, S], F32R, kind="ExternalInput")
    xv = nc.dram_tensor("xv", [E, S], F32R, kind="ExternalInput")
    wq = nc.dram_tensor("wq", [E, P], F32R, kind="ExternalInput")
    wk = nc.dram_tensor("wk", [E, P], F32R, kind="ExternalInput")
    wv = nc.dram_tensor("wv", [E, HL * (D + 1)], F32R, kind="ExternalInput")
    wo = nc.dram_tensor("wo", [P, E], F32R, kind="ExternalInput")
    bq = nc.dram_tensor("bq", [128, MT], F32, kind="ExternalInput")
    bk = nc.dram_tensor("bk", [128, MT], F32, kind="ExternalInput")
    bv = nc.dram_tensor("bv", [1, HL * (D + 1)], F32R, kind="ExternalInput")
    ones = nc.dram_tensor("ones", [1, 128], F32R, kind="ExternalInput")
    out = nc.dram_tensor("out", [E, S], F32, kind="ExternalOutput")

    with tile.TileContext(nc) as tc:
        with (
            tc.tile_pool(name="consts", bufs=1) as cpool,
            tc.tile_pool(name="xstage", bufs=3) as xpool,
            tc.tile_pool(name="kqv", bufs=1) as kqv_pool,
            tc.tile_pool(name="exp", bufs=4) as exp_pool,
            tc.tile_pool(name="attnsb", bufs=2) as attnsb_pool,
            tc.tile_pool(name="small", bufs=4) as small_pool,
            tc.tile_pool(name="outstage", bufs=3) as out_pool,
            tc.tile_pool(name="ps", bufs=2, space=bass.MemorySpace.PSUM) as ps_pool,
            tc.tile_pool(name="psattn", bufs=4, space=bass.MemorySpace.PSUM) as psa_pool,
        ):
            # ---- constants ----
            wq_sb = cpool.tile([128, EKT, P], F32R, tag="wq")
            wk_sb = cpool.tile([128, EKT, P], F32R, tag="wk")
            wv_sb = cpool.tile([128, EKT, HL * (D + 1)], F32R, tag="wv")
            wo_sb = cpool.tile([128, MT, E], F32R, tag="wo")
            bq_sb = cpool.tile([128, MT], F32, tag="bq")
            bk_sb = cpool.tile([128, MT], F32, tag="bk")
            bv_sb = cpool.tile([1, HL * (D + 1)], F32R, tag="bv")
            ones_row = cpool.tile([1, 128], F32R, tag="ones")

            nc.sync.dma_start(wq_sb[:], wq.ap().rearrange("(a p) m -> p a m", p=128))
            nc.sync.dma_start(wk_sb[:], wk.ap().rearrange("(a p) m -> p a m", p=128))
            nc.sync.dma_start(wv_sb[:], wv.ap().rearrange("(a p) m -> p a m", p=128))
            nc.sync.dma_start(wo_sb[:], wo.ap().rearrange("(a p) m -> p a m", p=128))
            nc.sync.dma_start(bq_sb[:], bq.ap())
            nc.sync.dma_start(bk_sb[:], bk.ap())
            nc.sync.dma_start(bv_sb[:], bv.ap())
            nc.sync.dma_start(ones_row[:], ones.ap())
            nc.gpsimd.load_library(library_config.attn)

            kT_sb = kqv_pool.tile([128, MT, S], F32R, tag="kT")
            qT_sb = kqv_pool.tile([128, MT, S], F32R, tag="qT")
            v_sb = kqv_pool.tile([128, NKT, HL, D + 1], F32R, tag="v")

            def load_chunk(x, sc2, tag):
                t = xpool.tile([128, EKT, 512], F32R, tag="x", name="x_" + tag)
                nc.sync.dma_start(
                    t[:], x.ap()[:, sc2 * 512:(sc2 + 1) * 512]
                    .rearrange("(a p) s -> p a s", p=128))
                return t

            def proj_kq(x_t, w_sb, b_sb, dst_sb, sc2):
                # dst[:, mt, sc2*512:...] = w.T @ x + b
                ps = ps_pool.tile([128, 2, 512], F32, tag="mm",
                                  name=f"proj_{sc2}")
                for mt in range(MT):
                    for ekt in range(EKT):
                        nc.tensor.matmul(
                            ps[:, mt, :],
                            _r(w_sb[:, ekt, mt * 128:(mt + 1) * 128]),
                            _r(x_t[:, ekt, :]),
                            start=(ekt == 0), stop=(ekt == EKT - 1))
                for mt in range(MT):
                    nc.vector.tensor_scalar_add(
                        dst_sb[:, mt, sc2 * 512:(sc2 + 1) * 512],
                        ps[:, mt, :], b_sb[:, mt:mt + 1])

            def proj_v(xv_t, sc2):
                # v[st, :] = xv.T @ wv + bv, st-tiles of 128 rows
                PV = HL * (D + 1)
                for stp in range(2):
                    ps = ps_pool.tile([128, 2, 512], F32, tag="mm",
                                      name=f"vproj_{sc2}_{stp}")
                    for i in range(2):
                        sti = 2 * stp + i
                        st = sc2 * 4 + sti
                        for ekt in range(EKT):
                            nc.tensor.matmul(
                                ps[:, i, 0:PV],
                                _r(xv_t[:, ekt, sti * 128:(sti + 1) * 128]),
                                _r(wv_sb[:, ekt, :]),
                                start=(ekt == 0), stop=False)
                        nc.tensor.matmul(
                            ps[:, i, 0:PV], _r(ones_row[:]), _r(bv_sb[:]),
                            start=False, stop=True)
                        nc.vector.tensor_copy(
                            v_sb[:, st, :, :],
                            ps[:, i, 0:PV].rearrange("p (h d) -> p h d", h=HL))

            # ---- phase B: k, v projections over all s-chunks ----
            for sc2 in range(NSC):
                xk_t = load_chunk(xk, sc2, "xk")
                xv_t = load_chunk(xv, sc2, "xv")
                proj_kq(xk_t, wk_sb, bk_sb, kT_sb, sc2)
                proj_v(xv_t, sc2)

            # ---- phase C: q projection + attention + out-proj, per q-chunk.
            # Scores for a head-pair land in one 2-bank psum tile, exp'd by a
            # single wide ScalarE op; PV runs one kt behind scores so the PE
            # never waits on ScalarE. Out-proj of chunk sc is deferred until
            # after the attention matmuls of chunk sc+1 are emitted, keeping
            # the PE stream dense across chunk boundaries.
            def emit_outproj(sc, attn_sb):
                for mtp in range(E // 256):
                    ps_o = ps_pool.tile([128, 2, 512], F32, tag="mm",
                                        name=f"pso_{sc}_{mtp}")
                    for i in range(2):
                        mt = 2 * mtp + i
                        for kt2 in range(MT):
                            nc.tensor.matmul(
                                ps_o[:, i, :],
                                _r(wo_sb[:, kt2, mt * 128:(mt + 1) * 128]),
                                _r(attn_sb[:, kt2, :]),
                                start=(kt2 == 0), stop=(kt2 == MT - 1))
                    ot = out_pool.tile([128, 2, 512], F32, tag="ot")
                    nc.vector.tensor_copy(ot[:], ps_o[:])
                    for i in range(2):
                        mt = 2 * mtp + i
                        nc.sync.dma_start(
                            out.ap()[mt * 128:(mt + 1) * 128,
                                     sc * 512:(sc + 1) * 512],
                            ot[:, i, :])

            pending = None
            for sc in range(NSC):
                xq_t = load_chunk(xq, sc, "xq")
                proj_kq(xq_t, wq_sb, bq_sb, qT_sb, sc)

                attn_sb = attnsb_pool.tile([128, MT, 512], F32R, tag="attn_sb")
                for hp in range(2):
                    attn_ps = {}
                    for i in range(2):
                        h = 2 * hp + i
                        attn_ps[h] = psa_pool.tile([D + 1, 512], F32, tag="pv",
                                                   name=f"pv_{sc}_{h}")
                    exp_tiles = {}

                    def emit_scores(kt):
                        s_ps = ps_pool.tile([128, 2, 512], F32, tag="mm",
                                            name=f"sps_{sc}_{hp}_{kt}")
                        for i in range(2):
                            lo, hi = i * 64, (i + 1) * 64
                            nc.tensor.matmul(
                                s_ps[:, i, :],
                                _r(kT_sb[lo:hi, hp, kt * 128:(kt + 1) * 128]),
                                _r(qT_sb[lo:hi, hp, sc * 512:(sc + 1) * 512]),
                                start=True, stop=True,
                                tile_position=(lo, 0) if ROW_PACK else None)
                        exp_t = exp_pool.tile([128, 2, 512], F32R, tag="exp",
                                              name=f"exp_{sc}_{hp}_{kt}")
                        nc.scalar.activation(
                            exp_t[:], s_ps[:],
                            mybir.ActivationFunctionType.Exp,
                            scale=0.125)
                        exp_tiles[kt] = exp_t

                    def emit_pv(kt):
                        exp_t = exp_tiles.pop(kt)
                        for i in range(2):
                            h = 2 * hp + i
                            nc.tensor.matmul(
                                attn_ps[h][:],
                                _r(v_sb[:, kt, h, :]),
                                _r(exp_t[:, i, :]),
                                start=(kt == 0), stop=(kt == NKT - 1))

                    for kt in range(NKT):
                        emit_scores(kt)
                        if kt > 0:
                            emit_pv(kt - 1)
                    emit_pv(NKT - 1)

                    for i in range(2):
                        h = 2 * hp + i
                        rc = small_pool.tile([1, 512], F32, tag="recip")
                        nc.vector.reciprocal(rc[:], attn_ps[h][D:D + 1, :])
                        bc = small_pool.tile([D, 512], F32, tag="bc")
                        nc.gpsimd.partition_broadcast(bc[:], rc[:])
                        nc.vector.tensor_mul(
                            attn_sb[(h % 2) * 64:(h % 2 + 1) * 64, h // 2, :],
                            attn_ps[h][0:D, :], bc[:])

                if pending is not None:
                    emit_outproj(*pending)
                pending = (sc, attn_sb)
            emit_outproj(*pending)

    nc.compile()
    return nc


_NC_CACHE = None


def _get_nc():
    global _NC_CACHE
    if _NC_CACHE is None:
        _NC_CACHE = build_nc()
    return _NC_CACHE


def make_in_maps(key, query, value, Wk, bk, Wq, bq, Wv, bv, Wo, bo):
    key = np.asarray(key, np.float32)
    query = np.asarray(query, np.float32)
    value = np.asarray(value, np.float32)
    in_maps = []
    xqT = [np.ascontiguousarray(query[b].T) for b in range(2)]
    xkT = [np.ascontiguousarray(key[b].T) for b in range(2)]
    xvT = [np.ascontiguousarray(value[b].T) for b in range(2)]
    for c in range(NCORES):
        b, g = divmod(c, 4)
        rows = slice(g * P, (g + 1) * P)
        wv_slice = np.asarray(Wv, np.float32)[rows].T  # [E, 256]
        bv_slice = np.asarray(bv, np.float32)[rows]
        wv_ext = np.zeros((E, HL * (D + 1)), np.float32)
        bv_ext = np.zeros((1, HL * (D + 1)), np.float32)
        for h in range(HL):
            wv_ext[:, h * (D + 1):h * (D + 1) + D] = wv_slice[:, h * D:(h + 1) * D]
            bv_ext[0, h * (D + 1):h * (D + 1) + D] = bv_slice[h * D:(h + 1) * D]
            bv_ext[0, h * (D + 1) + D] = 1.0
        in_maps.append({
            "xq": xqT[b],
            "xk": xkT[b],
            "xv": xvT[b],
            "wq": np.ascontiguousarray(np.asarray(Wq, np.float32)[rows].T),
            "wk": np.ascontiguousarray(np.asarray(Wk, np.float32)[rows].T),
            "wv": wv_ext,
            "wo": np.ascontiguousarray(np.asarray(Wo, np.float32)[:, rows].T),
            "bq": np.ascontiguousarray(
                np.asarray(bq, np.float32)[rows].reshape(MT, 128).T),
            "bk": np.ascontiguousarray(
                np.asarray(bk, np.float32)[rows].reshape(MT, 128).T),
            "bv": bv_ext,
            "ones": np.ones((1, 128), np.float32),
        })
    return in_maps


def assemble(results, bo):
    bo = np.asarray(bo, np.float32)
    out = np.empty((2, S, E), np.float32)
    for b in range(2):
        acc = results[4 * b]["out"].astype(np.float32).copy()
        for g in range(1, 4):
            acc += results[4 * b + g]["out"]
        out[b] = acc.T + bo[None, :]
    return out


def kernel(key, query, value, Wk, bk, Wq, bq, Wv, bv, Wo, bo):
    from concourse.bass_utils import run_bass_kernel_spmd

    nc = _get_nc()
    in_maps = make_in_maps(key, query, value, Wk, bk, Wq, bq, Wv, bv, Wo, bo)
    trace = os.environ.get("KB_TRACE", "0") == "1"
    kwargs = {}
    if trace:
        kwargs["trace"] = True
        kwargs["trace_cores"] = list(range(NCORES))
    res = run_bass_kernel_spmd(nc, in_maps, core_ids=list(range(NCORES)), **kwargs)
    if trace:
        kernel.last_results = res
    return assemble(res.results, bo)


# revision 9
# speedup vs baseline: 1.2453x; 1.0948x over previous
"""Multi-head attention (unfused) for one TRN2 chip (8 NeuronCores).

Sharding: 2 batches x 4 head-groups (4 heads each) = 8 cores.
Core c handles batch b = c // 4, head-group g = c % 4 (heads 4g..4g+3,
i.e. rows 256g..256g+255 of the QKV projections).

Host side pre-transposes activations to [E, S] ("xT") and weights so the
device kernel never transposes anything:
  qT = WqT.T @ xqT + bq          [256, S]   (lhsT=WqT tile, rhs=xqT tile)
  kT = WkT.T @ xkT + bk          [256, S]
  v  = xvT.T @ WvT + bv          [S, 256]   (lhsT=xvT tile, rhs=WvT tile)
  per head h:
    scoresT = kT_h.T @ qT_h      [S_k, S_q] (layout: keys on partitions)
    expT    = exp(scoresT/8)     (ScalarE, scale fused)
    pv      = [v_h | 1].T @ expT [65, S_q]  (row 64 = softmax denominator)
    attnT_h = pv[0:64] * (1/pv[64]) (broadcast via stride-0 DMA)
  outT_partial = WoT.T @ attnT   [E, S]
Host sums the 4 partials per batch, adds bo, transposes back.

All matmuls run with float32r operands (full-rate PE) accumulating fp32.
"""

import os
import sys

sys.path.insert(0, "/opt/trn_rl_repo")

import numpy as np

import concourse.bacc as bacc
import concourse.bass as bass
import concourse.mybir as mybir
import concourse.tile as tile
from concourse import library_config

F32 = mybir.dt.float32
F32R = mybir.dt.float32r

S = 2048          # sequence length (keys and queries)
E = 1024          # embed dim
P = 256           # projection rows per core (4 heads x 64)
D = 64            # head dim
HL = 4            # heads per core
NCORES = 8

EKT = E // 128    # 8 contraction k-tiles for projections
MT = P // 128     # 2 m-tiles for kT/qT
NSC = S // 512    # 4 s-chunks / q-chunks
NKT = S // 128    # 16 key tiles

ROW_PACK = os.environ.get("KB_ROW_PACK", "1") == "1"


def _r(ap):
    return ap.bitcast(F32R)


def build_nc():
    nc = bacc.Bacc(trn_type="TRN2", debug=False, num_devices=NCORES,
                   enable_asserts=False)

    xq = nc.dram_tensor("xq", [E, S], F32R, kind="ExternalInput")
    xk = nc.dram_tensor("xk", [E, S], F32R, kind="ExternalInput")
    xv = nc.dram_tensor("xv", [E, S], F32R, kind="ExternalInput")
    wq = nc.dram_tensor("wq", [E, P], F32R, kind="ExternalInput")
    wk = nc.dram_tensor("wk", [E, P], F32R, kind="ExternalInput")
    wv = nc.dram_tensor("wv", [E, HL * (D + 1)], F32R, kind="ExternalInput")
    wo = nc.dram_tensor("wo", [P, E], F32R, kind="ExternalInput")
    bq = nc.dram_tensor("bq", [128, MT], F32, kind="ExternalInput")
    bk = nc.dram_tensor("bk", [128, MT], F32, kind="ExternalInput")
    bv = nc.dram_tensor("bv", [1, HL * (D + 1)], F32R, kind="ExternalInput")
    ones = nc.dram_tensor("ones", [1, 128], F32R, kind="ExternalInput")
    out = nc.dram_tensor("out", [E, S], F32, kind="ExternalOutput")

    with tile.TileContext(nc) as tc:
        with (
            tc.tile_pool(name="consts", bufs=1) as cpool,
            tc.tile_pool(name="xstage", bufs=3) as xpool,
            tc.tile_pool(name="kqv", bufs=1) as kqv_pool,
            tc.tile_pool(name="exp", bufs=4) as exp_pool,
            tc.tile_pool(name="attnsb", bufs=2) as attnsb_pool,
            tc.tile_pool(name="pvsb", bufs=2) as pvsb_pool,
            tc.tile_pool(name="small", bufs=4) as small_pool,
            tc.tile_pool(name="outstage", bufs=3) as out_pool,
            tc.tile_pool(name="ps", bufs=3, space=bass.MemorySpace.PSUM) as ps_pool,
            tc.tile_pool(name="psattn", bufs=2, space=bass.MemorySpace.PSUM) as psa_pool,
        ):
            # ---- constants ----
            wq_sb = cpool.tile([128, EKT, P], F32R, tag="wq")
            wk_sb = cpool.tile([128, EKT, P], F32R, tag="wk")
            wv_sb = cpool.tile([128, EKT, HL * (D + 1)], F32R, tag="wv")
            wo_sb = cpool.tile([128, MT, E], F32R, tag="wo")
            bq_sb = cpool.tile([128, MT], F32, tag="bq")
            bk_sb = cpool.tile([128, MT], F32, tag="bk")
            bv_sb = cpool.tile([1, HL * (D + 1)], F32R, tag="bv")
            ones_row = cpool.tile([1, 128], F32R, tag="ones")

            nc.sync.dma_start(wq_sb[:], wq.ap().rearrange("(a p) m -> p a m", p=128))
            nc.sync.dma_start(wk_sb[:], wk.ap().rearrange("(a p) m -> p a m", p=128))
            nc.sync.dma_start(wv_sb[:], wv.ap().rearrange("(a p) m -> p a m", p=128))
            nc.sync.dma_start(wo_sb[:], wo.ap().rearrange("(a p) m -> p a m", p=128))
            nc.sync.dma_start(bq_sb[:], bq.ap())
            nc.sync.dma_start(bk_sb[:], bk.ap())
            nc.sync.dma_start(bv_sb[:], bv.ap())
            nc.sync.dma_start(ones_row[:], ones.ap())
            nc.gpsimd.load_library(library_config.attn)

            kT_sb = kqv_pool.tile([128, MT, S], F32R, tag="kT")
            qT_sb = kqv_pool.tile([128, MT, S], F32R, tag="qT")
            v_sb = kqv_pool.tile([128, NKT, HL, D + 1], F32R, tag="v")

            def load_chunk(x, sc2, tag):
                t = xpool.tile([128, EKT, 512], F32R, tag="x", name="x_" + tag)
                nc.sync.dma_start(
                    t[:], x.ap()[:, sc2 * 512:(sc2 + 1) * 512]
                    .rearrange("(a p) s -> p a s", p=128))
                return t

            def proj_kq(x_t, w_sb, b_sb, dst_sb, sc2):
                # dst[:, mt, sc2*512:...] = w.T @ x + b
                ps = ps_pool.tile([128, 2, 512], F32, tag="mm",
                                  name=f"proj_{sc2}")
                for mt in range(MT):
                    for ekt in range(EKT):
                        nc.tensor.matmul(
                            ps[:, mt, :],
                            _r(w_sb[:, ekt, mt * 128:(mt + 1) * 128]),
                            _r(x_t[:, ekt, :]),
                            start=(ekt == 0), stop=(ekt == EKT - 1))
                for mt in range(MT):
                    nc.vector.tensor_scalar_add(
                        dst_sb[:, mt, sc2 * 512:(sc2 + 1) * 512],
                        ps[:, mt, :], b_sb[:, mt:mt + 1])

            def proj_v(xv_t, sc2):
                # v[st, :] = xv.T @ wv + bv, st-tiles of 128 rows
                PV = HL * (D + 1)
                for stp in range(2):
                    ps = ps_pool.tile([128, 2, 512], F32, tag="mm",
                                      name=f"vproj_{sc2}_{stp}")
                    for i in range(2):
                        sti = 2 * stp + i
                        st = sc2 * 4 + sti
                        for ekt in range(EKT):
                            nc.tensor.matmul(
                                ps[:, i, 0:PV],
                                _r(xv_t[:, ekt, sti * 128:(sti + 1) * 128]),
                                _r(wv_sb[:, ekt, :]),
                                start=(ekt == 0), stop=False)
                        nc.tensor.matmul(
                            ps[:, i, 0:PV], _r(ones_row[:]), _r(bv_sb[:]),
                            start=False, stop=True)
                        nc.vector.tensor_copy(
                            v_sb[:, st, :, :],
                            ps[:, i, 0:PV].rearrange("p (h d) -> p h d", h=HL))

            # ---- phase B: k, v projections over all s-chunks ----
            for sc2 in range(NSC):
                xk_t = load_chunk(xk, sc2, "xk")
                xv_t = load_chunk(xv, sc2, "xv")
                proj_kq(xk_t, wk_sb, bk_sb, kT_sb, sc2)
                proj_v(xv_t, sc2)

            # ---- phase C: q projection + attention + out-proj, per q-chunk.
            # Scores for a head-pair land in one 2-bank psum tile, exp'd by a
            # single wide ScalarE op; PV runs one kt behind scores so the PE
            # never waits on ScalarE. Out-proj of chunk sc is deferred until
            # after the attention matmuls of chunk sc+1 are emitted, keeping
            # the PE stream dense across chunk boundaries.
            def emit_outproj(sc, attn_sb):
                for mtp in range(E // 256):
                    ps_o = ps_pool.tile([128, 2, 512], F32, tag="mm",
                                        name=f"pso_{sc}_{mtp}")
                    for i in range(2):
                        mt = 2 * mtp + i
                        for kt2 in range(MT):
                            nc.tensor.matmul(
                                ps_o[:, i, :],
                                _r(wo_sb[:, kt2, mt * 128:(mt + 1) * 128]),
                                _r(attn_sb[:, kt2, :]),
                                start=(kt2 == 0), stop=(kt2 == MT - 1))
                    ot = out_pool.tile([128, 2, 512], F32, tag="ot")
                    nc.vector.tensor_copy(ot[:], ps_o[:])
                    for i in range(2):
                        mt = 2 * mtp + i
                        nc.sync.dma_start(
                            out.ap()[mt * 128:(mt + 1) * 128,
                                     sc * 512:(sc + 1) * 512],
                            ot[:, i, :])

            pending = None
            for sc in range(NSC):
                xq_t = load_chunk(xq, sc, "xq")
                proj_kq(xq_t, wq_sb, bq_sb, qT_sb, sc)

                attn_sb = attnsb_pool.tile([128, MT, 512], F32R, tag="attn_sb")
                for hp in range(2):
                    attn_ps = {}
                    for i in range(2):
                        h = 2 * hp + i
                        attn_ps[h] = psa_pool.tile([D + 1, 512], F32, tag="pv",
                                                   name=f"pv_{sc}_{h}")
                    exp_tiles = {}

                    def emit_scores(kt):
                        s_ps = ps_pool.tile([128, 2, 512], F32, tag="mm",
                                            name=f"sps_{sc}_{hp}_{kt}")
                        for i in range(2):
                            lo, hi = i * 64, (i + 1) * 64
                            nc.tensor.matmul(
                                s_ps[:, i, :],
                                _r(kT_sb[lo:hi, hp, kt * 128:(kt + 1) * 128]),
                                _r(qT_sb[lo:hi, hp, sc * 512:(sc + 1) * 512]),
                                start=True, stop=True,
                                tile_position=(lo, 0) if ROW_PACK else None)
                        exp_t = exp_pool.tile([128, 2, 512], F32R, tag="exp",
                                              name=f"exp_{sc}_{hp}_{kt}")
                        nc.scalar.activation(
                            exp_t[:], s_ps[:],
                            mybir.ActivationFunctionType.Exp,
                            scale=0.125)
                        exp_tiles[kt] = exp_t

                    def emit_pv(kt):
                        exp_t = exp_tiles.pop(kt)
                        for i in range(2):
                            h = 2 * hp + i
                            nc.tensor.matmul(
                                attn_ps[h][:],
                                _r(v_sb[:, kt, h, :]),
                                _r(exp_t[:, i, :]),
                                start=(kt == 0), stop=(kt == NKT - 1))

                    for kt in range(NKT):
                        emit_scores(kt)
                        if kt > 1:
                            emit_pv(kt - 2)
                    emit_pv(NKT - 2)
                    emit_pv(NKT - 1)

                    # evacuate PV psum quickly (frees banks for the next
                    # head-pair), normalize later from SBUF off the PE path
                    pv_sb = pvsb_pool.tile([D + 1, 2, 512], F32, tag="pv_sb",
                                           name=f"pvsb_{sc}_{hp}")
                    for i in range(2):
                        h = 2 * hp + i
                        nc.vector.tensor_copy(pv_sb[:, i, :], attn_ps[h][:])
                    for i in range(2):
                        h = 2 * hp + i
                        rc = small_pool.tile([1, 512], F32, tag="recip")
                        nc.vector.reciprocal(rc[:], pv_sb[D:D + 1, i, :])
                        bc = small_pool.tile([D, 512], F32, tag="bc")
                        nc.gpsimd.partition_broadcast(bc[:], rc[:])
                        nc.vector.tensor_mul(
                            attn_sb[(h % 2) * 64:(h % 2 + 1) * 64, h // 2, :],
                            pv_sb[0:D, i, :], bc[:])

                if pending is not None:
                    emit_outproj(*pending)
                pending = (sc, attn_sb)
            emit_outproj(*pending)

    nc.compile()
    return nc


_NC_CACHE = None


def _get_nc():
    global _NC_CACHE
    if _NC_CACHE is None:
        _NC_CACHE = build_nc()
    return _NC_CACHE


def make_in_maps(key, query, value, Wk, bk, Wq, bq, Wv, bv, Wo, bo):
    key = np.asarray(key, np.float32)
    query = np.asarray(query, np.float32)
    value = np.asarray(value, np.float32)
    in_maps = []
    xqT = [np.ascontiguousarray(query[b].T) for b in range(2)]
    xkT = [np.ascontiguousarray(key[b].T) for b in range(2)]
    xvT = [np.ascontiguousarray(value[b].T) for b in range(2)]
    for c in range(NCORES):
        b, g = divmod(c, 4)
        rows = slice(g * P, (g + 1) * P)
        wv_slice = np.asarray(Wv, np.float32)[rows].T  # [E, 256]
        bv_slice = np.asarray(bv, np.float32)[rows]
        wv_ext = np.zeros((E, HL * (D + 1)), np.float32)
        bv_ext = np.zeros((1, HL * (D + 1)), np.float32)
        for h in range(HL):
            wv_ext[:, h * (D + 1):h * (D + 1) + D] = wv_slice[:, h * D:(h + 1) * D]
            bv_ext[0, h * (D + 1):h * (D + 1) + D] = bv_slice[h * D:(h + 1) * D]
            bv_ext[0, h * (D + 1) + D] = 1.0
        in_maps.append({
            "xq": xqT[b],
            "xk": xkT[b],
            "xv": xvT[b],
            "wq": np.ascontiguousarray(np.asarray(Wq, np.float32)[rows].T),
            "wk": np.ascontiguousarray(np.asarray(Wk, np.float32)[rows].T),
            "wv": wv_ext,
            "wo": np.ascontiguousarray(np.asarray(Wo, np.float32)[:, rows].T),
            "bq": np.ascontiguousarray(
                np.asarray(bq, np.float32)[rows].reshape(MT, 128).T),
            "bk": np.ascontiguousarray(
                np.asarray(bk, np.float32)[rows].reshape(MT, 128).T),
            "bv": bv_ext,
            "ones": np.ones((1, 128), np.float32),
        })
    return in_maps


def assemble(results, bo):
    bo = np.asarray(bo, np.float32)
    out = np.empty((2, S, E), np.float32)
    for b in range(2):
        acc = results[4 * b]["out"].astype(np.float32).copy()
        for g in range(1, 4):
            acc += results[4 * b + g]["out"]
        out[b] = acc.T + bo[None, :]
    return out


def kernel(key, query, value, Wk, bk, Wq, bq, Wv, bv, Wo, bo):
    from concourse.bass_utils import run_bass_kernel_spmd

    nc = _get_nc()
    in_maps = make_in_maps(key, query, value, Wk, bk, Wq, bq, Wv, bv, Wo, bo)
    trace = os.environ.get("KB_TRACE", "0") == "1"
    kwargs = {}
    if trace:
        kwargs["trace"] = True
        kwargs["trace_cores"] = list(range(NCORES))
    res = run_bass_kernel_spmd(nc, in_maps, core_ids=list(range(NCORES)), **kwargs)
    if trace:
        kernel.last_results = res
    return assemble(res.results, bo)
